# revision 9
# baseline (speedup 1.0000x reference)
"""GAT layer (gnn_message_passing) Trainium2 Bass kernel.

Problem (hardcoded): B=4 graphs, N=8192 nodes, K=16 neighbors, D=128, H=4
heads, DK=32.  8 NeuronCores: core = graph*2 + node-half, each core owns
4096 nodes of one graph.

Per-core algorithm:
  1. PE projects Q/K/V densely from host-transposed X^T (project once per
     vertex, NOT per gathered copy).  K,V packed per-node into a DRAM row;
     Q kept in SBUF.
  2. dma_gather fetches the 17 (self + 16 neighbor) packed KV rows per node
     into [128-node, 17, 2D] SBUF tiles  (the memory-bound part).
  3. DVE/ACT compute scores = q.k per head (mul + segmented reduce), masked
     softmax (no max subtraction needed: scores bounded, mask -> exp==0),
     and probs*V products.
  4. PE transpose-accumulates the 17 product slices into ctx^T (PSUM
     accumulation), which directly feeds the transposed-orientation
     MLP chain (Wo -> W1+relu -> W2) with per-partition ACT biases,
     then transposes back and DMAs out.
"""

import functools
import os
import sys

import numpy as np

if "/opt/trn_rl_repo" not in sys.path:
    sys.path.insert(0, "/opt/trn_rl_repo")

B, N, K, D, H = 4, 8192, 16, 128, 4
DK = D // H
S = K + 1                 # neighbor slots incl. self
NCORES = 8
HALF = N // 2             # nodes per core
NBLK = HALF // 128        # attention blocks per core (32)
PBLK = N // 128           # projection blocks per core (64)
IDXC = (S * 128) // 16    # idx columns per block (136)
MASK_NEG = -1e9

# Gather payload dtype: bf16 rows are 512B (2x faster gather), f32 are 1KB.
KV_BF16 = os.environ.get("GAT_KV_F32", "0") != "1"


def _build_nc():
    import concourse.bacc as bacc
    import concourse.bass as bass
    import concourse.tile as tile
    from concourse import mybir
    from concourse.library_config import mlp as mlp_lib
    from concourse.masks import make_identity
    from concourse.tile_rust import add_dep_helper

    KVDT = mybir.dt.bfloat16 if KV_BF16 else mybir.dt.float32
    F32 = mybir.dt.float32
    AF = mybir.ActivationFunctionType
    OP = mybir.AluOpType

    nc = bacc.Bacc("TRN2", debug=False)

    # ---- DRAM I/O ----
    xt = nc.dram_tensor("xt", [D, N], F32, kind="ExternalInput")
    xt_own = nc.dram_tensor("xt_own", [D, HALF], F32, kind="ExternalInput")
    wq = nc.dram_tensor("wq", [D, D], F32, kind="ExternalInput")
    wk = nc.dram_tensor("wk", [D, D], F32, kind="ExternalInput")
    wv = nc.dram_tensor("wv", [D, D], F32, kind="ExternalInput")
    wo = nc.dram_tensor("wo", [D, D], F32, kind="ExternalInput")
    w1 = nc.dram_tensor("w1", [D, D], F32, kind="ExternalInput")
    w2 = nc.dram_tensor("w2", [D, D], F32, kind="ExternalInput")
    bq = nc.dram_tensor("bq", [D], F32, kind="ExternalInput")
    bk = nc.dram_tensor("bk", [D], F32, kind="ExternalInput")
    bv = nc.dram_tensor("bv", [D], F32, kind="ExternalInput")
    bo = nc.dram_tensor("bo", [D], F32, kind="ExternalInput")
    b1 = nc.dram_tensor("b1", [D], F32, kind="ExternalInput")
    b2 = nc.dram_tensor("b2", [D], F32, kind="ExternalInput")
    # partition-major preswizzled idx / mask (see host prep)
    idx = nc.dram_tensor("idx", [128, NBLK * IDXC], mybir.dt.int16,
                         kind="ExternalInput")
    mask = nc.dram_tensor("mask", [128, NBLK * S], F32, kind="ExternalInput")
    kv_dram = nc.dram_tensor("kv_scratch", [N, 2 * D], KVDT)  # Internal
    y = nc.dram_tensor("y", [HALF, D], F32, kind="ExternalOutput")

    with tile.TileContext(nc) as tc:
        import contextlib
        ctx = contextlib.ExitStack()
        with ctx:
            nc.gpsimd.load_library(mlp_lib)
            persist = ctx.enter_context(tc.tile_pool(name="persist", bufs=1))
            # --- persistent SBUF state ---
            w_sb = {}
            for name, dr in [("wq", wq), ("wk", wk), ("wv", wv),
                             ("wo", wo), ("w1", w1), ("w2", w2)]:
                t = persist.tile([D, D], F32, tag=f"w_{name}")
                nc.sync.dma_start(out=t[:], in_=dr.ap())
                w_sb[name] = t
            # bq/bk/bv replicated across partitions ([128, D])
            brep = {}
            for name, dr in [("bq", bq), ("bk", bk), ("bv", bv)]:
                t = persist.tile([128, D], F32, tag=f"brep_{name}")
                src = bass.AP(tensor=dr, offset=0, ap=[[0, 128], [1, D]])
                nc.sync.dma_start(out=t[:], in_=src)
                brep[name] = t
            # bo/b1/b2 as per-partition columns ([128, 1])
            bcol = {}
            for name, dr in [("bo", bo), ("b1", b1), ("b2", b2)]:
                t = persist.tile([128, 1], F32, tag=f"bcol_{name}")
                src = bass.AP(tensor=dr, offset=0, ap=[[1, 128], [0, 1]])
                nc.sync.dma_start(out=t[:], in_=src)
                bcol[name] = t
            ident_kv = persist.tile([128, 128], KVDT, tag="ident_kv")
            make_identity(nc, ident_kv[:])
            ident_f32 = persist.tile([128, 128], F32, tag="ident_f32")
            make_identity(nc, ident_f32[:])
            idx_sb = persist.tile([128, NBLK * IDXC], mybir.dt.int16,
                                  tag="idx")
            nc.sync.dma_start(out=idx_sb[:], in_=idx.ap())
            mask_sb = persist.tile([128, NBLK * S], F32, tag="mask")
            nc.sync.dma_start(out=mask_sb[:], in_=mask.ap())
            q_all = persist.tile([128, NBLK * 128], KVDT, tag="q_all")

            # ---- phase 1: projections ----
            kv_writes = []
            with tc.tile_pool(name="xt_pool", bufs=1) as xpool, \
                 tc.tile_pool(name="proj_sbuf", bufs=3) as psb, \
                 tc.tile_pool(name="proj_psum", bufs=2, space="PSUM") as pps:
                xt_sb = xpool.tile([D, N], F32, tag="xt")
                nc.sync.dma_start(out=xt_sb[:], in_=xt.ap())
                xt_own_sb = xpool.tile([D, HALF], F32, tag="xt_own")
                nc.sync.dma_start(out=xt_own_sb[:], in_=xt_own.ap())
                for b in range(PBLK):
                    xts = xt_sb[:, b * 128:(b + 1) * 128]
                    kp = pps.tile([128, D], F32, space="PSUM", tag="kp")
                    vp = pps.tile([128, D], F32, space="PSUM", tag="vp")
                    nc.tensor.matmul(out=kp[:], lhsT=xts, rhs=w_sb["wk"][:],
                                     start=True, stop=True)
                    nc.tensor.matmul(out=vp[:], lhsT=xts, rhs=w_sb["wv"][:],
                                     start=True, stop=True)
                    kvt = psb.tile([128, 2 * D], KVDT, tag="kvt")
                    nc.vector.scalar_tensor_tensor(
                        out=kvt[:, 0:D], in0=kp[:], scalar=1.0,
                        in1=brep["bk"][:], op0=OP.mult, op1=OP.add)
                    nc.vector.scalar_tensor_tensor(
                        out=kvt[:, D:2 * D], in0=vp[:], scalar=1.0,
                        in1=brep["bv"][:], op0=OP.mult, op1=OP.add)
                    wi = nc.sync.dma_start(
                        out=kv_dram.ap()[b * 128:(b + 1) * 128, :],
                        in_=kvt[:])
                    kv_writes.append(wi)
                for b in range(NBLK):
                    xts = xt_own_sb[:, b * 128:(b + 1) * 128]
                    qp = pps.tile([128, D], F32, space="PSUM", tag="qp")
                    nc.tensor.matmul(out=qp[:], lhsT=xts, rhs=w_sb["wq"][:],
                                     start=True, stop=True)
                    nc.vector.scalar_tensor_tensor(
                        out=q_all[:, b * 128:(b + 1) * 128], in0=qp[:],
                        scalar=1.0, in1=brep["bq"][:], op0=OP.mult,
                        op1=OP.add)

            # barrier: every gather must wait for all kv writes
            barrier = nc.vector.engine_nop()
            for wi_ in kv_writes:
                add_dep_helper(barrier.ins, wi_.ins, reason="kv ready")

            # ---- phase 2: gather + attention + MLP ----
            with tc.tile_pool(name="khv", bufs=3) as khv_pool, \
                 tc.tile_pool(name="attn", bufs=2) as attn_pool, \
                 tc.tile_pool(name="ctx_ps", bufs=2, space="PSUM") as ctx_ps, \
                 tc.tile_pool(name="mlp_ps", bufs=4, space="PSUM") as mlp_ps, \
                 tc.tile_pool(name="out_ps", bufs=2, space="PSUM") as out_ps:
                for t in range(NBLK):
                    khv = khv_pool.tile([128, S, 2 * D], KVDT, tag="khv")
                    gi = nc.gpsimd.dma_gather(
                        khv[:, :, :], kv_dram.ap(),
                        idx_sb[:, t * IDXC:(t + 1) * IDXC],
                        S * 128, S * 128, 2 * D, single_packet=False)
                    add_dep_helper(gi.ins, barrier.ins, reason="kv ready")

                    # scores: prod_k = k * q  -> segmented reduce over DK
                    prod_k = attn_pool.tile([128, S, D], KVDT, tag="prod_k")
                    q_bc = q_all[:, t * 128:(t + 1) * 128] \
                        .rearrange("p (o d) -> p o d", o=1) \
                        .to_broadcast([128, S, D])
                    nc.vector.tensor_tensor(
                        out=prod_k[:], in0=khv[:, :, 0:D], in1=q_bc,
                        op=OP.mult)
                    scores = attn_pool.tile([128, S * H], F32, tag="scores")
                    nc.vector.tensor_reduce(
                        out=scores[:].rearrange("p (s h) -> p s h", h=H),
                        in_=prod_k[:].rearrange("p s (h k) -> p s h k", h=H),
                        axis=mybir.AxisListType.X, op=OP.add)
                    # scaled scores + mask
                    scores2 = attn_pool.tile([128, S * H], F32, tag="scores2")
                    m_bc = mask_sb[:, t * S:(t + 1) * S] \
                        .rearrange("p (s o) -> p s o", o=1) \
                        .to_broadcast([128, S, H])
                    nc.vector.scalar_tensor_tensor(
                        out=scores2[:].rearrange("p (s h) -> p s h", h=H),
                        in0=scores[:].rearrange("p (s h) -> p s h", h=H),
                        scalar=float(1.0 / np.sqrt(DK)), in1=m_bc,
                        op0=OP.mult, op1=OP.add)
                    probs = attn_pool.tile([128, S * H], KVDT, tag="probs")
                    nc.scalar.activation(out=probs[:], in_=scores2[:],
                                         func=AF.Exp)
                    ssum = attn_pool.tile([128, H], F32, tag="ssum")
                    nc.vector.tensor_reduce(
                        out=ssum[:],
                        in_=probs[:].rearrange("p (s h) -> p h s", h=H),
                        axis=mybir.AxisListType.X, op=OP.add)
                    rinv = attn_pool.tile([128, H], F32, tag="rinv")
                    nc.vector.reciprocal(rinv[:], ssum[:])
                    rinv_kv = attn_pool.tile([128, H], KVDT, tag="rinv_kv")
                    nc.vector.tensor_copy(rinv_kv[:], rinv[:])
                    probs_n = attn_pool.tile([128, S * H], KVDT, tag="probs_n")
                    r_bc = rinv_kv[:].rearrange("p (o h) -> p o h", o=1) \
                        .to_broadcast([128, S, H])
                    nc.vector.tensor_tensor(
                        out=probs_n[:].rearrange("p (s h) -> p s h", h=H),
                        in0=probs[:].rearrange("p (s h) -> p s h", h=H),
                        in1=r_bc, op=OP.mult)
                    # expand probs over DK on ACT, then weight V
                    probs_e = attn_pool.tile([128, S, D], KVDT, tag="probs_e")
                    p_src = probs_n[:] \
                        .rearrange("p (s h o) -> p s h o", h=H, o=1) \
                        .to_broadcast([128, S, H, DK])
                    nc.scalar.activation(out=probs_e[:].rearrange(
                        "p s (h k) -> p s h k", h=H),
                        in_=p_src, func=AF.Copy)
                    prod_v = attn_pool.tile([128, S, D], KVDT, tag="prod_v")
                    nc.vector.tensor_tensor(
                        out=prod_v[:], in0=khv[:, :, D:2 * D],
                        in1=probs_e[:], op=OP.mult)
                    # ctx^T accumulation on PE
                    ctxT = ctx_ps.tile([128, 128], F32, space="PSUM",
                                       tag="ctxT")
                    for s_ in range(S):
                        nc.tensor.matmul(out=ctxT[:],
                                         lhsT=prod_v[:, s_, :],
                                         rhs=ident_kv[:],
                                         start=(s_ == 0), stop=(s_ == S - 1))
                    ctxT_sb = attn_pool.tile([128, 128], F32, tag="ctxT_sb")
                    nc.scalar.copy(ctxT_sb[:], ctxT[:])
                    # MLP in transposed orientation
                    y1 = mlp_ps.tile([128, 128], F32, space="PSUM", tag="mlp")
                    nc.tensor.matmul(out=y1[:], lhsT=w_sb["wo"][:],
                                     rhs=ctxT_sb[:], start=True, stop=True)
                    s1 = attn_pool.tile([128, 128], F32, tag="s1")
                    nc.scalar.activation(out=s1[:], in_=y1[:],
                                         func=AF.Identity,
                                         bias=bcol["bo"][:], scale=1.0)
                    y2 = mlp_ps.tile([128, 128], F32, space="PSUM", tag="mlp")
                    nc.tensor.matmul(out=y2[:], lhsT=w_sb["w1"][:],
                                     rhs=s1[:], start=True, stop=True)
                    s2 = attn_pool.tile([128, 128], F32, tag="s2")
                    nc.scalar.activation(out=s2[:], in_=y2[:], func=AF.Relu,
                                         bias=bcol["b1"][:], scale=1.0)
                    y3 = mlp_ps.tile([128, 128], F32, space="PSUM", tag="mlp")
                    nc.tensor.matmul(out=y3[:], lhsT=w_sb["w2"][:],
                                     rhs=s2[:], start=True, stop=True)
                    s3 = attn_pool.tile([128, 128], F32, tag="s3")
                    nc.scalar.activation(out=s3[:], in_=y3[:],
                                         func=AF.Identity,
                                         bias=bcol["b2"][:], scale=1.0)
                    outT = out_ps.tile([128, 128], F32, space="PSUM",
                                       tag="outT")
                    nc.tensor.matmul(out=outT[:], lhsT=s3[:],
                                     rhs=ident_f32[:], is_transpose=True,
                                     start=True, stop=True)
                    out_sb = attn_pool.tile([128, 128], F32, tag="out_sb")
                    nc.vector.tensor_copy(out_sb[:], outT[:])
                    nc.sync.dma_start(
                        out=y.ap()[t * 128:(t + 1) * 128, :], in_=out_sb[:])
    nc.compile()
    return nc


@functools.lru_cache(maxsize=1)
def _get_nc():
    return _build_nc()


def _host_prep(vertex_feat, neighbors_idx, valid_lens,
               Wq, bq, Wk, bk, Wv, bv, Wo, bo, W1, b1, W2, b2):
    """Build the 8 per-core input maps."""
    vf = np.asarray(vertex_feat, np.float32)
    nbr = np.asarray(neighbors_idx)
    vl = np.asarray(valid_lens)
    shared = {
        "wq": np.ascontiguousarray(np.asarray(Wq, np.float32).reshape(D, D)),
        "wk": np.ascontiguousarray(np.asarray(Wk, np.float32).reshape(D, D)),
        "wv": np.ascontiguousarray(np.asarray(Wv, np.float32).reshape(D, D)),
        "wo": np.ascontiguousarray(np.asarray(Wo, np.float32).reshape(D, D)),
        "w1": np.ascontiguousarray(np.asarray(W1, np.float32)),
        "w2": np.ascontiguousarray(np.asarray(W2, np.float32)),
        "bq": np.asarray(bq, np.float32).reshape(D).copy(),
        "bk": np.asarray(bk, np.float32).reshape(D).copy(),
        "bv": np.asarray(bv, np.float32).reshape(D).copy(),
        "bo": np.asarray(bo, np.float32).reshape(D).copy(),
        "b1": np.asarray(b1, np.float32).reshape(D).copy(),
        "b2": np.asarray(b2, np.float32).reshape(D).copy(),
    }
    in_maps = []
    for core in range(NCORES):
        g, half = core // 2, core % 2
        lo = half * HALF
        xt_g = np.ascontiguousarray(vf[g].T)                 # [D, N]
        xt_own = np.ascontiguousarray(vf[g, lo:lo + HALF].T)  # [D, HALF]
        # gather indices: j = s*128 + p  ->  partition j%16, col j//16
        own_nbr = nbr[g, lo:lo + HALF].astype(np.int32)       # [HALF, K]
        jflat = np.empty((NBLK, S * 128), np.int16)
        node_ids = (lo + np.arange(HALF)).astype(np.int16).reshape(NBLK, 128)
        for t in range(NBLK):
            jf = jflat[t].reshape(S, 128)
            jf[0] = node_ids[t]
            jf[1:] = own_nbr[t * 128:(t + 1) * 128].T.astype(np.int16)
        idx16 = jflat.reshape(NBLK, IDXC, 16)                # [t, col, p16]
        idx16 = np.transpose(idx16, (2, 0, 1))               # [p16, t, col]
        idx_pm = np.tile(idx16, (8, 1, 1))                   # [128, t, col]
        idx_pm = np.ascontiguousarray(idx_pm.reshape(128, NBLK * IDXC))
        # additive mask [HALF, S] -> partition-major [128, NBLK*S]
        own_vl = vl[g, lo:lo + HALF].astype(np.int32)
        pos = np.arange(S)[None, :]
        m = np.where(pos < (own_vl + 1)[:, None], 0.0,
                     MASK_NEG).astype(np.float32)             # [HALF, S]
        m = m.reshape(NBLK, 128, S).transpose(1, 0, 2)        # [128, t, S]
        mask_pm = np.ascontiguousarray(m.reshape(128, NBLK * S))
        in_maps.append(dict(shared, xt=xt_g, xt_own=xt_own,
                            idx=idx_pm, mask=mask_pm))
    return in_maps


def kernel(**inputs):
    from concourse.bass_utils import run_bass_kernel_spmd

    in_maps = _host_prep(**inputs)
    nc = _get_nc()
    res = run_bass_kernel_spmd(nc, in_maps, list(range(NCORES)))
    out = np.empty((B, N, D), np.float32)
    for core in range(NCORES):
        g, half = core // 2, core % 2
        out[g, half * HALF:(half + 1) * HALF] = res.results[core]["y"]
    return out
